# revision 1
# baseline (speedup 1.0000x reference)
"""Single-head causal attention on 8 Trainium2 NeuronCores.

Problem: x[8, 4096, 384], Wq/Wk/Wv[384, 64] ->
    out[b] = softmax(causal((x[b]Wq)(x[b]Wk)^T / sqrt(384))) @ (x[b]Wv)

Sharding: data-parallel over batch — core i computes batch element i.
Weights are replicated to every core.

Per-core kernel layout (all matmuls contract over the partition axis):
  - X^T tiles [c=128, t] are built from natural x tiles via PE transposes.
  - Q^T, K^T [64, T] = W^T X^T  (lhsT = W chunk [128c, 64], rhs = X^T).
    Both are stored twice (partitions 0:64 and 64:128) so score matmuls
    can be row-packed two-at-a-time into the 128x128 PE array.
  - V_ext [t=128, 65] = [X Wv | 1]  (ones column -> softmax denominator).
  - Scores are computed TRANSPOSED: S^T[s, q] = K Q^T so that the
    softmax sum over s becomes a matmul-friendly partition axis and
    P^T tiles feed the PV matmul with no per-tile transposes:
        O^T[h+1, q] += V_ext[s,:]^T @ P^T[s, q]   (row 64 = sum_s P)
  - exp via ScalarE activation (no max subtraction: |scores/sqrt(C)| is
    small for this distribution, exp cannot overflow in fp32).
  - Causal masking: multiply diagonal-block P^T tiles by one of four
    precomputed 0/1 masks (built once with gpsimd.affine_select).
  - O^T is PE-transposed back to [q=128, 65]; column 64 holds the row
    sums; divide and DMA out.
"""

import sys

if "/opt/trn_rl_repo" not in sys.path:
    sys.path.insert(0, "/opt/trn_rl_repo")

import numpy as np

import concourse.bass as bass  # noqa: F401  (AP types used implicitly)
import concourse.tile as tile
from concourse import bacc, mybir
from concourse.bass import ds
from concourse.bass_utils import run_bass_kernel_spmd
from concourse.masks import make_identity

B = 8
T_FULL = 4096
C = 384
H = 64
P = 128
TQ = 512  # q-block width
SCALE = 1.0 / float(np.sqrt(C))
F32 = mybir.dt.float32
F32R = mybir.dt.float32r

F16 = mybir.dt.float16
MM_DTYPE = F16  # matmul pipeline dtype (fp16: 1 cyc/row + fast weight load)
ROW_PACK = True  # run score matmuls two-at-a-time in PE row halves


def build_nc(T=T_FULL, mm_dtype=MM_DTYPE, row_pack=ROW_PACK):
    """Build the per-core Bass program (same program on all 8 cores)."""
    NT = T // P  # number of 128-row s-chunks
    NQ = T // TQ  # number of 512-row q-blocks
    CC = C // P  # 3 embed chunks
    SUB = TQ // P  # 4 sub-tiles per block

    MMD = mm_dtype  # tiles feeding matmuls are allocated in this dtype

    def mm_cast(ap):
        return ap

    nc = bacc.Bacc(
        "TRN2",
        target_bir_lowering=False,
        debug=False,
        enable_asserts=True,
        num_devices=B,
    )
    x_ap = nc.dram_tensor("x", [T, C], F32, kind="ExternalInput").ap()
    wq_ap = nc.dram_tensor("Wq", [C, H], F32, kind="ExternalInput").ap()
    wk_ap = nc.dram_tensor("Wk", [C, H], F32, kind="ExternalInput").ap()
    wv_ap = nc.dram_tensor("Wv", [C, H], F32, kind="ExternalInput").ap()
    out_ap = nc.dram_tensor("out", [T, H], F32, kind="ExternalOutput").ap()

    x_re = x_ap.rearrange("(n p) c -> p n c", p=P)  # [128, NT, 384]
    out_re = out_ap.rearrange("(n p) h -> p n h", p=P)  # [128, NT, 64]

    with tile.TileContext(nc) as tc:
        with (
            tc.tile_pool(name="consts", bufs=1) as consts,
            tc.tile_pool(name="xnat", bufs=3) as xnat,
            tc.tile_pool(name="xtp", bufs=3) as xtp,
            tc.tile_pool(name="qkt", bufs=1) as qktp,
            tc.tile_pool(name="vextp", bufs=1) as vextp,
            tc.tile_pool(name="ptp", bufs=4) as ptp,
            tc.tile_pool(name="otp", bufs=2) as otp,
            tc.tile_pool(name="vtp", bufs=2) as vtp,
            tc.tile_pool(name="op", bufs=2) as op_,
            tc.tile_pool(name="rvp", bufs=2) as rvp,
            tc.tile_pool(name="psum", bufs=2, space="PSUM") as psum,
        ):
            ident = consts.tile([P, P], F32)
            make_identity(nc, ident)
            ident_h = consts.tile([P, P], MMD)
            make_identity(nc, ident_h)
            wq_sb = consts.tile([P, CC, H], MMD)
            nc.gpsimd.dma_start(out=wq_sb, in_=wq_ap.rearrange("(c p) h -> p c h", p=P))
            wk_sb = consts.tile([P, CC, H], MMD)
            nc.gpsimd.dma_start(out=wk_sb, in_=wk_ap.rearrange("(c p) h -> p c h", p=P))
            wv_sb = consts.tile([P, CC, H], MMD)
            nc.gpsimd.dma_start(out=wv_sb, in_=wv_ap.rearrange("(c p) h -> p c h", p=P))

            # masks[d][s_local, q_local] = 1.0 where q_local - s_local - 128*d >= 0
            masks = consts.tile([P, SUB, TQ], MMD)
            nc.vector.memset(masks, 1.0)
            for d in range(SUB):
                nc.gpsimd.affine_select(
                    out=masks[:, d, :],
                    in_=masks[:, d, :],
                    compare_op=mybir.AluOpType.is_ge,
                    fill=0.0,
                    base=-P * d,
                    pattern=[[1, TQ]],
                    channel_multiplier=-1,
                )

            if row_pack:
                # qt2: Q^T duplicated in both partition halves.
                # kt2: K^T chunk c lives at partitions 64*(c%2), col (c//2)*128.
                qt2 = qktp.tile([P, T], MMD, tag="qt")
                kt2 = qktp.tile([P, (NT // 2) * P], MMD, tag="kt")
            else:
                qt2 = qktp.tile([H, T], MMD, tag="qt")
                kt2 = qktp.tile([H, T], MMD, tag="kt")
            vext = vextp.tile([P, NT, H + 1], MMD)
            ones_col = consts.tile([P, NT, 1], F32)
            nc.vector.memset(ones_col, 1.0)
            nc.vector.tensor_copy(out=vext[:, :, H : H + 1], in_=ones_col)

            def phase1_gen(j):
                """Load x rows [512j, 512j+512), produce X^T, Q^T, K^T, V.

                Yields between small PE chunks so the driver can spread
                this work into the gaps of the ScalarE-bound attention
                pair loop without ever blocking the pss->exp pipeline
                (phase-1 PSUM lives on the "acc" tag, not "wide").
                """
                xn = xnat.tile([P, SUB, C], MMD, tag="xn", name=f"xn{j}")
                nc.gpsimd.dma_start(out=xn, in_=x_re[:, SUB * j : SUB * (j + 1), :])
                xt = xtp.tile([P, CC, TQ], MMD, tag="xt", name=f"xt{j}")
                yield
                for st in range(SUB):
                    pst = psum.tile([P, CC, P], MMD, tag="small", name=f"pst{j}_{st}")
                    for c in range(CC):
                        nc.tensor.transpose(
                            pst[:, c, :], xn[:, st, c * P : (c + 1) * P], ident_h
                        )
                    nc.vector.tensor_copy(
                        out=xt[:, :, st * P : (st + 1) * P], in_=pst
                    )
                    yield
                blk = ds(j * TQ, TQ)
                psq = psum.tile([H, TQ], F32, tag="acc", name=f"psq{j}")
                for c in range(CC):
                    nc.tensor.matmul(
                        psq,
                        lhsT=wq_sb[:, c, :],
                        rhs=xt[:, c, :],
                        start=(c == 0),
                        stop=(c == CC - 1),
                    )
                if row_pack:
                    nc.vector.tensor_copy(out=qt2[0:H, blk], in_=psq)
                    nc.vector.tensor_copy(out=qt2[H:P, blk], in_=psq)
                else:
                    nc.vector.tensor_copy(out=qt2[:, blk], in_=psq)
                yield
                psk = psum.tile([H, TQ], F32, tag="acc", name=f"psk{j}")
                for c in range(CC):
                    nc.tensor.matmul(
                        psk,
                        lhsT=wk_sb[:, c, :],
                        rhs=xt[:, c, :],
                        start=(c == 0),
                        stop=(c == CC - 1),
                    )
                if row_pack:
                    for st in range(SUB):
                        c = SUB * j + st
                        half = H * (c % 2)
                        nc.vector.tensor_copy(
                            out=kt2[half : half + H, (c // 2) * P : (c // 2 + 1) * P],
                            in_=psk[:, st * P : (st + 1) * P],
                        )
                else:
                    nc.vector.tensor_copy(out=kt2[:, blk], in_=psk)
                yield
                psv = psum.tile([H, TQ], F32, tag="acc", name=f"psv{j}")
                for c in range(CC):
                    nc.tensor.matmul(
                        psv,
                        lhsT=wv_sb[:, c, :],
                        rhs=xt[:, c, :],
                        start=(c == 0),
                        stop=(c == CC - 1),
                    )
                vt = vtp.tile([H, TQ], MMD, tag="vt", name=f"vt{j}")
                nc.vector.tensor_copy(out=vt, in_=psv)
                yield
                for st in range(SUB):
                    pvt = psum.tile([P, H], MMD, tag="small", name=f"pvt{j}_{st}")
                    nc.tensor.transpose(
                        pvt, vt[:, st * P : (st + 1) * P], ident_h[0:H, 0:H]
                    )
                    nc.vector.tensor_copy(
                        out=vext[:, SUB * j + st, 0:H], in_=pvt
                    )
                yield

            N1_CHUNKS = 9

            def phase2(j, pump):
                """Attention for q rows [512j, 512j+512).  pump(done, total)
                advances the interleaved next-block phase-1 generator."""
                nchunks = (j + 1) * SUB
                q_sl = ds(j * TQ, TQ)
                npairs = nchunks // 2
                pso = psum.tile([H + 1, TQ], F32, tag="acc", name=f"pso{j}")
                for pr in range(npairs):
                    pss = psum.tile([P, 2 * TQ], F32, tag="wide", name=f"pss{j}_{pr}")
                    for h2 in range(2):
                        c = 2 * pr + h2
                        if row_pack:
                            half = H * (c % 2)
                            nc.tensor.matmul(
                                pss[:, h2 * TQ : (h2 + 1) * TQ],
                                lhsT=kt2[half : half + H, (c // 2) * P : (c // 2 + 1) * P],
                                rhs=qt2[half : half + H, q_sl],
                                start=True,
                                stop=True,
                                tile_position=(half, 0),
                            )
                        else:
                            nc.tensor.matmul(
                                pss[:, h2 * TQ : (h2 + 1) * TQ],
                                lhsT=kt2[:, c * P : (c + 1) * P],
                                rhs=qt2[:, q_sl],
                                start=True,
                                stop=True,
                            )
                    pt = ptp.tile([P, 2 * TQ], MMD, tag="pt", name=f"pt{j}_{pr}")
                    nc.scalar.activation(
                        out=pt,
                        in_=pss,
                        func=mybir.ActivationFunctionType.Exp,
                        scale=SCALE,
                    )
                    for h2 in range(2):
                        c = 2 * pr + h2
                        d = c - SUB * j
                        if d >= 0:
                            nc.vector.tensor_mul(
                                out=pt[:, h2 * TQ : (h2 + 1) * TQ],
                                in0=pt[:, h2 * TQ : (h2 + 1) * TQ],
                                in1=masks[:, d, :],
                            )
                    for h2 in range(2):
                        c = 2 * pr + h2
                        nc.tensor.matmul(
                            pso,
                            lhsT=vext[:, c, :],
                            rhs=pt[:, h2 * TQ : (h2 + 1) * TQ],
                            start=(c == 0),
                            stop=(c == nchunks - 1),
                        )
                    pump(pr + 1, npairs)
                ot = otp.tile([H + 1, TQ], MMD, tag="ot", name=f"ot{j}")
                nc.vector.tensor_copy(out=ot, in_=pso)
                pstr = psum.tile([P, SUB, H + 2], MMD, tag="small", name=f"pstr{j}")
                for i in range(SUB):
                    nc.tensor.transpose(
                        pstr[:, i, 0 : H + 1],
                        ot[:, i * P : (i + 1) * P],
                        ident_h[0 : H + 1, 0 : H + 1],
                    )
                o = op_.tile([P, SUB, H + 1], F32, tag="o", name=f"o{j}")
                nc.vector.tensor_copy(out=o, in_=pstr[:, :, 0 : H + 1])
                rv = rvp.tile([P, SUB], F32, tag="rv", name=f"rv{j}")
                nc.vector.reciprocal(out=rv, in_=o[:, :, H : H + 1])
                for i in range(SUB):
                    nc.vector.tensor_scalar_mul(
                        out=o[:, i, 0:H],
                        in0=o[:, i, 0:H],
                        scalar1=rv[:, i : i + 1],
                    )
                nc.sync.dma_start(
                    out=out_re[:, SUB * j : SUB * (j + 1), :], in_=o[:, :, 0:H]
                )

            for j in range(min(2, NQ)):
                for _ in phase1_gen(j):
                    pass
            for j in range(NQ):
                gen = phase1_gen(j + 2) if j + 2 < NQ else None
                adv = {"n": 0}

                def pump(done, total, gen=gen, adv=adv):
                    if gen is None:
                        return
                    want = done * N1_CHUNKS // total
                    while adv["n"] < want:
                        try:
                            next(gen)
                        except StopIteration:
                            break
                        adv["n"] += 1

                phase2(j, pump)
                if gen is not None:
                    for _ in gen:
                        pass

    nc.compile()
    return nc


_NC_CACHE = {}


def _get_nc():
    if "nc" not in _NC_CACHE:
        _NC_CACHE["nc"] = build_nc()
    return _NC_CACHE["nc"]


def kernel(x, Wk, Wq, Wv, _trace=False, _trace_kwargs=None):
    x = np.ascontiguousarray(x, dtype=np.float32)
    Wk = np.ascontiguousarray(Wk, dtype=np.float32)
    Wq = np.ascontiguousarray(Wq, dtype=np.float32)
    Wv = np.ascontiguousarray(Wv, dtype=np.float32)
    nc = _get_nc()
    in_maps = [
        {"x": x[b], "Wq": Wq, "Wk": Wk, "Wv": Wv} for b in range(B)
    ]
    res = run_bass_kernel_spmd(
        nc, in_maps, list(range(B)), trace=_trace, **(_trace_kwargs or {})
    )
    out = np.stack([res.results[b]["out"] for b in range(B)], axis=0)
    if _trace:
        return out, res
    return out



# revision 44
# speedup vs baseline: 1.6327x; 1.6327x over previous
"""Single-head causal attention on 8 Trainium2 NeuronCores.

Problem: x[8, 4096, 384], Wq/Wk/Wv[384, 64] ->
    out[b] = softmax(causal((x[b]Wq)(x[b]Wk)^T / sqrt(384))) @ (x[b]Wv)

Sharding: data-parallel over batch — core i computes batch element i.
Weights are replicated to every core.

Per-core kernel layout (all matmuls contract over the partition axis):
  - X^T tiles [c=128, t] are built from natural x tiles via PE transposes.
  - Q^T and K^T [64, T] are produced together: lhsT = [Wq | Wk] packed
    [128c, 128] so one matmul chain yields PSUM [128, 512] with Q^T in
    partitions 0:64 and K^T in 64:128.
  - V is produced in natural orientation [t=128, 64] directly
    (lhsT = X^T chunk, rhs = Wv chunk) and stored as V_ext = [V | 1]
    (ones column -> softmax denominator).
  - Scores are computed TRANSPOSED: S^T[s, q] = K Q^T so that the
    softmax sum over s becomes a matmul-friendly partition axis and
    P^T tiles feed the PV matmul with no per-tile transposes:
        O^T[h+1, q] += V_ext[s,:]^T @ P^T[s, q]   (row 64 = sum_s P)
  - exp via ScalarE activation (no max subtraction: |scores/sqrt(C)| is
    small for this distribution, exp cannot overflow in fp32).
  - Diagonal-block chunks are NARROWED: chunk d of block j only covers
    q columns [128d, 512) (the rest is fully masked), cutting ~8% of
    score/PV matmul columns and exp elements. Within the narrowed
    region only the leading [128, 128] triangle needs masking, done
    in-place with gpsimd affine_select (Pool engine, otherwise idle).
  - O^T is PE-transposed back to [q=128, 65]; column 64 holds the row
    sums; divide and DMA out.
"""

import sys

if "/opt/trn_rl_repo" not in sys.path:
    sys.path.insert(0, "/opt/trn_rl_repo")

import numpy as np

import concourse.bass as bass  # noqa: F401  (AP types used implicitly)
import concourse.tile as tile
from concourse import bacc, mybir
from concourse.bass import ds
from concourse.bass_utils import run_bass_kernel_spmd
from concourse.masks import make_identity

B = 8
T_FULL = 4096
C = 384
H = 64
P = 128
TQ = 512  # q-block width
SCALE = 1.0 / float(np.sqrt(C))
F32 = mybir.dt.float32

F16 = mybir.dt.float16
MM_DTYPE = F16  # matmul pipeline dtype (fp16: 1 cyc/row + fast weight load)


def build_nc(T=T_FULL, mm_dtype=MM_DTYPE):
    """Build the per-core Bass program (same program on all 8 cores)."""
    NT = T // P  # number of 128-row s-chunks
    NQ = T // TQ  # number of 512-row q-blocks
    CC = C // P  # 3 embed chunks
    SUB = TQ // P  # 4 sub-tiles per block

    MMD = mm_dtype  # tiles feeding matmuls are allocated in this dtype

    nc = bacc.Bacc(
        "TRN2",
        target_bir_lowering=False,
        debug=False,
        enable_asserts=True,
        num_devices=B,
    )
    x_ap = nc.dram_tensor("x", [T, C], F32, kind="ExternalInput").ap()
    wq_ap = nc.dram_tensor("Wq", [C, H], F32, kind="ExternalInput").ap()
    wk_ap = nc.dram_tensor("Wk", [C, H], F32, kind="ExternalInput").ap()
    wv_ap = nc.dram_tensor("Wv", [C, H], F32, kind="ExternalInput").ap()
    out_ap = nc.dram_tensor("out", [T, H], F32, kind="ExternalOutput").ap()

    x_re = x_ap.rearrange("(n p) c -> p n c", p=P)  # [128, NT, 384]
    out_re = out_ap.rearrange("(n p) h -> p n h", p=P)  # [128, NT, 64]

    with tile.TileContext(nc) as tc:
        with (
            tc.tile_pool(name="consts", bufs=1) as consts,
            tc.tile_pool(name="xnat", bufs=4) as xnat,
            tc.tile_pool(name="xtp", bufs=3) as xtp,
            tc.tile_pool(name="qkt", bufs=1) as qktp,
            tc.tile_pool(name="vextp", bufs=1) as vextp,
            tc.tile_pool(name="ptp", bufs=4) as ptp,
            tc.tile_pool(name="otp", bufs=2) as otp,
            tc.tile_pool(name="op", bufs=2) as op_,
            tc.tile_pool(name="rvp", bufs=2) as rvp,
            tc.tile_pool(name="psum", bufs=2, space="PSUM") as psum,
        ):
            ident_h = consts.tile([P, P], MMD)
            # packed QK weights: chunk c -> [Wq_c | Wk_c]  [128, 128]
            wqk_sb = consts.tile([P, CC, P], MMD)
            wv_sb = consts.tile([P, CC, H], MMD)
            qt2 = qktp.tile([H, T], MMD, tag="qt")
            kt2 = qktp.tile([H, T], MMD, tag="kt")
            vext = vextp.tile([P, NT, H + 1], MMD)
            ones_col = consts.tile([P, NT, 1], F32)
            # causal triangle mask: tri[s, q] = 1.0 where q >= s (128x128)
            tri = consts.tile([P, P], MMD)

            def emit_consts():
                make_identity(nc, ident_h)
                nc.gpsimd.dma_start(
                    out=wqk_sb[:, :, 0:H],
                    in_=wq_ap.rearrange("(c p) h -> p c h", p=P),
                )
                nc.gpsimd.dma_start(
                    out=wqk_sb[:, :, H:P],
                    in_=wk_ap.rearrange("(c p) h -> p c h", p=P),
                )
                nc.gpsimd.dma_start(
                    out=wv_sb, in_=wv_ap.rearrange("(c p) h -> p c h", p=P)
                )
                nc.vector.memset(ones_col, 1.0)
                nc.vector.tensor_copy(out=vext[:, :, H : H + 1], in_=ones_col)
                nc.vector.memset(tri, 1.0)
                nc.gpsimd.affine_select(
                    out=tri,
                    in_=tri,
                    compare_op=mybir.AluOpType.is_ge,
                    fill=0.0,
                    base=0,
                    pattern=[[1, P]],
                    channel_multiplier=-1,
                )

            def p1copy(j, out, in_, qk=False):
                # block 0's Q/K copies go on the still-idle ScalarE so DVE
                # can race ahead on the X^T copies (ramp latency).
                if qk and j <= 2:
                    nc.scalar.copy(out=out, in_=in_)
                else:
                    nc.vector.tensor_copy(out=out, in_=in_)

            def phase1_gen(j):
                """Load x rows [512j, 512j+512), produce X^T, Q^T, K^T, V.

                Yields between small PE chunks so the driver can spread
                this work into the gaps of the attention pair loop.
                """
                xn = xnat.tile([P, SUB, C], MMD, tag="xn", name=f"xn{j}")
                nc.gpsimd.dma_start(
                    out=xn[:, 0:2, :], in_=x_re[:, SUB * j : SUB * j + 2, :]
                )
                nc.gpsimd.dma_start(
                    out=xn[:, 2:4, :], in_=x_re[:, SUB * j + 2 : SUB * (j + 1), :]
                )
                xt = xtp.tile([P, CC, TQ], MMD, tag="xt", name=f"xt{j}")
                yield
                for st in range(SUB):
                    pst = psum.tile([P, CC, P], MMD, tag="small", name=f"pst{j}_{st}")
                    for c in range(CC):
                        nc.tensor.transpose(
                            pst[:, c, :], xn[:, st, c * P : (c + 1) * P], ident_h
                        )
                    p1copy(j, xt[:, :, st * P : (st + 1) * P], pst)
                    yield
                blk = ds(j * TQ, TQ)
                psqk = psum.tile([P, TQ], F32, tag="acc", name=f"psqk{j}")
                for c in range(CC):
                    nc.tensor.matmul(
                        psqk,
                        lhsT=wqk_sb[:, c, :],
                        rhs=xt[:, c, :],
                        start=(c == 0),
                        stop=(c == CC - 1),
                    )
                p1copy(j, qt2[:, blk], psqk[0:H, :], qk=True)
                yield
                p1copy(j, kt2[:, blk], psqk[H:P, :], qk=True)
                yield
                for st in range(SUB):
                    psvn = psum.tile([P, H], F32, tag="acc", name=f"psvn{j}_{st}")
                    for c in range(CC):
                        nc.tensor.matmul(
                            psvn,
                            lhsT=xt[:, c, st * P : (st + 1) * P],
                            rhs=wv_sb[:, c, :],
                            start=(c == 0),
                            stop=(c == CC - 1),
                        )
                    nc.vector.tensor_copy(
                        out=vext[:, SUB * j + st, 0:H], in_=psvn
                    )
                    yield

            N1_CHUNKS = 11  # number of yields in phase1_gen

            def make_finish(j, psoj):
                def finish(last=(j == NQ - 1)):
                    ot = otp.tile([H + 1, TQ], MMD, tag="ot", name=f"ot{j}")
                    if last:
                        # ScalarE is idle once the final exp retires
                        nc.scalar.copy(out=ot, in_=psoj)
                    else:
                        nc.vector.tensor_copy(out=ot, in_=psoj)
                    pstr = psum.tile(
                        [P, SUB, H + 2], MMD, tag="small", name=f"pstr{j}"
                    )
                    for i in range(SUB):
                        nc.tensor.transpose(
                            pstr[:, i, 0 : H + 1],
                            ot[:, i * P : (i + 1) * P],
                            ident_h[0 : H + 1, 0 : H + 1],
                        )
                    o = op_.tile([P, SUB, H + 1], F32, tag="o", name=f"o{j}")
                    nc.vector.tensor_copy(out=o, in_=pstr[:, :, 0 : H + 1])
                    rv = rvp.tile([P, SUB], F32, tag="rv", name=f"rv{j}")
                    nc.vector.reciprocal(out=rv, in_=o[:, :, H : H + 1])
                    for i in range(SUB):
                        eng = nc.gpsimd if (last and i >= 2) else nc.vector
                        eng.tensor_scalar_mul(
                            out=o[:, i, 0:H],
                            in0=o[:, i, 0:H],
                            scalar1=rv[:, i : i + 1],
                        )
                    if last:
                        # final block: split the store across two queues so
                        # the tail isn't serialized behind one DMA
                        nc.sync.dma_start(
                            out=out_re[:, SUB * j : SUB * j + 2, :],
                            in_=o[:, 0:2, 0:H],
                        )
                        nc.scalar.dma_start(
                            out=out_re[:, SUB * j + 2 : SUB * (j + 1), :],
                            in_=o[:, 2:4, 0:H],
                        )
                    else:
                        nc.sync.dma_start(
                            out=out_re[:, SUB * j : SUB * (j + 1), :],
                            in_=o[:, :, 0:H],
                        )

                return finish

            # ---- Global chunk-stream attention ----
            # Every (block, s-chunk) score/exp/PV unit joins one continuous
            # stream of chunk PAIRS; a pair may span a block boundary, so
            # ScalarE sees a gapless exp stream with no per-block drain.
            # pso accumulators of adjacent blocks overlap (acc tag, 2 bufs)
            # and each block's output path is deferred by one pair so it
            # never stalls the following block's score matmuls.
            chunks = [(j, c) for j in range(NQ) for c in range(SUB * (j + 1))]
            pairs = [tuple(chunks[i : i + 2]) for i in range(0, len(chunks), 2)]
            pair_block = [pr[0][0] for pr in pairs]
            group_start = {}
            group_len = {}
            for pi, b in enumerate(pair_block):
                group_start.setdefault(b, pi)
                group_len[b] = group_len.get(b, 0) + 1

            gens = [phase1_gen(j) for j in range(NQ)]
            left = [N1_CHUNKS] * NQ  # chunks remaining per generator

            def advance(g):
                if g >= NQ or left[g] <= 0:
                    return
                try:
                    next(gens[g])
                except StopIteration:
                    left[g] = 0
                    return
                left[g] -= 1

            def drain(g):
                while g < NQ and left[g] > 0:
                    advance(g)

            advance(0)  # issue block 0's x DMA before const init
            emit_consts()
            advance(1)  # pre-issue the next three blocks' x DMAs
            advance(2)
            advance(3)
            drain(0)
            # pre-advance the next blocks' transpose chains: their x data
            # arrives during block 0's first pairs, and PE has slack there
            while left[1] > N1_CHUNKS - 7:
                advance(1)
            while left[2] > N1_CHUNKS - 5:
                advance(2)

            pso = {}
            pending = [None]
            req_quota = {}  # req gen chunk count at its pacing-group start
            pstate = {"pi": 0}

            def _pace(budget):
                # advance pending phase-1 toward its deadline, at most
                # `budget` chunks (keeps PE bursts small so score matmuls
                # are never delayed long, which would starve ScalarE)
                pi = pstate["pi"]
                cur = pair_block[pi]
                req, opp = cur + 1, cur + 2
                for g in (req, opp):
                    if g < NQ and left[g] == N1_CHUNKS:
                        advance(g)
                done = pi - group_start[cur] + 1
                total = group_len[cur]
                if req < NQ and left[req] > 0:
                    if req not in req_quota:
                        req_quota[req] = left[req]
                    want = -(-req_quota[req] * done // total)  # ceil
                    while (
                        left[req] > req_quota[req] - want
                        and left[req] > 0
                        and budget > 0
                    ):
                        advance(req)
                        budget -= 1
                if opp < NQ and left[opp] > 0 and budget > 0:
                    if opp not in req_quota:
                        req_quota[opp] = left[opp]
                    want = req_quota[opp] * done // (2 * total)
                    while (
                        left[opp] > req_quota[opp] - want
                        and left[opp] > 0
                        and budget > 0
                    ):
                        advance(opp)
                        budget -= 1

            def pump1():
                _pace(4)

            def pump2():
                _pace(6)

            for pi, pair in enumerate(pairs):
                pstate["pi"] = pi
                cur = pair_block[pi]
                for j, c in pair:
                    if c == 0:
                        drain(j)  # phase-1 of block j must be complete
                        pso[j] = psum.tile(
                            [H + 1, TQ], F32, tag="acc", name=f"pso{j}"
                        )
                pss = psum.tile([P, 2 * TQ], F32, tag="wide", name=f"pss{pi}")
                geoms = []
                base = 0
                for j, c in pair:
                    d = c - SUB * j
                    off = P * d if d >= 0 else 0
                    w = TQ - off
                    nc.tensor.matmul(
                        pss[:, base : base + w],
                        lhsT=kt2[:, c * P : (c + 1) * P],
                        rhs=qt2[:, j * TQ + off : (j + 1) * TQ],
                        start=True,
                        stop=True,
                    )
                    geoms.append((j, c, off, w, base, d >= 0))
                    base += w
                pt = ptp.tile([P, 2 * TQ], MMD, tag="pt", name=f"pt{pi}")
                nc.scalar.activation(
                    out=pt[:, 0:base],
                    in_=pss[:, 0:base],
                    func=mybir.ActivationFunctionType.Exp,
                    scale=SCALE,
                )
                # causal triangle mask on the leading 128 cols of each
                # diagonal chunk's region (keep where q_local >= s_local)
                for j, c, off, w, b, diag in geoms:
                    if diag:
                        nc.vector.tensor_mul(
                            out=pt[:, b : b + P],
                            in0=pt[:, b : b + P],
                            in1=tri,
                        )
                pump1()
                newly_done = None
                for j, c, off, w, b, diag in geoms:
                    nch = SUB * (j + 1)
                    nc.tensor.matmul(
                        pso[j][:, off:TQ],
                        lhsT=vext[:, c, :],
                        rhs=pt[:, b : b + w],
                        start=(c == 0),
                        stop=(c == nch - 1),
                    )
                    if c == nch - 1:
                        newly_done = j
                # emit the one-pair-deferred output path BEFORE pumping, so
                # its pso buffer frees before pumped phase-1 wants an acc slot
                if pending[0] is not None:
                    pending[0]()
                    pending[0] = None
                if newly_done is not None:
                    pending[0] = make_finish(newly_done, pso.pop(newly_done))
                # pace phase-1: gen cur+1 must finish by its group start,
                # gen cur+2 advances at half rate (deferred to pump2 below)
                pump2()
            if pending[0] is not None:
                pending[0]()

    nc.compile()
    return nc


_NC_CACHE = {}


def _get_nc():
    if "nc" not in _NC_CACHE:
        _NC_CACHE["nc"] = build_nc()
    return _NC_CACHE["nc"]


def kernel(x, Wk, Wq, Wv, _trace=False, _trace_kwargs=None):
    x = np.ascontiguousarray(x, dtype=np.float32)
    Wk = np.ascontiguousarray(Wk, dtype=np.float32)
    Wq = np.ascontiguousarray(Wq, dtype=np.float32)
    Wv = np.ascontiguousarray(Wv, dtype=np.float32)
    nc = _get_nc()
    in_maps = [
        {"x": x[b], "Wq": Wq, "Wk": Wk, "Wv": Wv} for b in range(B)
    ]
    res = run_bass_kernel_spmd(
        nc, in_maps, list(range(B)), trace=_trace, **(_trace_kwargs or {})
    )
    out = np.stack([res.results[b]["out"] for b in range(B)], axis=0)
    if _trace:
        return out, res
    return out


# revision 66
# speedup vs baseline: 1.6611x; 1.0174x over previous
"""Single-head causal attention on 8 Trainium2 NeuronCores.

Problem: x[8, 4096, 384], Wq/Wk/Wv[384, 64] ->
    out[b] = softmax(causal((x[b]Wq)(x[b]Wk)^T / sqrt(384))) @ (x[b]Wv)

Sharding: data-parallel over batch — core i computes batch element i.
Weights are replicated to every core.

Per-core kernel (all matmuls contract over the partition axis):
  - X^T tiles [c=128, t] are built from natural x tiles via PE transposes.
  - Q^T and K^T [64, T] are produced together: lhsT = [Wq | Wk] packed
    [128c, 128] so one matmul chain yields PSUM [128, 512] with Q^T in
    partitions 0:64 and K^T in 64:128 (one PSUM->SBUF copy each).
  - V is produced in natural orientation [t=128, 64] directly
    (lhsT = X^T chunk, rhs = Wv chunk) and stored as V_ext = [V | 1]
    (ones column -> softmax denominator via the PV matmul).
  - Scores are computed TRANSPOSED: S^T[s, q] = K Q^T so the softmax
    sum over s is a matmul-friendly partition axis and P^T tiles feed
    the PV matmul with no per-tile transposes:
        O^T[h+1, q] += V_ext[s,:]^T @ P^T[s, q]   (row 64 = sum_s P)
  - exp via ScalarE activation (no max subtraction: |scores/sqrt(C)| is
    small for this distribution, exp cannot overflow in fp32).
  - Diagonal-block chunks are NARROWED: chunk d of q-block j covers only
    q columns [128d, 512) (the rest is fully masked), cutting ~8% of
    score/PV columns and exp elements.  Within the narrowed region only
    the leading [128, 128] triangle needs masking - an in-place DVE
    multiply with a precomputed 0/1 triangle tile.
  - O^T is PE-transposed back to [q=128, 65]; column 64 holds the row
    sums; divide and DMA out.

Schedule: ScalarE's exp stream is the throughput floor (~61 us of
element time at 1 elem/cycle/lane), so everything is organized to keep
it gapless:
  - All (block, s-chunk) score/exp/PV units form ONE global stream of
    chunk pairs; a pair may span a q-block boundary, so there is no
    per-block pipeline drain.  pso accumulators of adjacent blocks
    overlap (acc tag, 2 PSUM bufs) and each block's output path is
    deferred by one pair so it never delays the next block's scores.
  - Phase-1 (x load, X^T, Q/K/V) for upcoming blocks is paced into the
    pair stream with deadlines (gen j+1 finishes exactly when block j+1
    starts; j+2.. advance at geometrically lower rates), in small
    budgeted steps so PE bursts never starve ScalarE.
  - Early blocks' Q/K copies run on ScalarE (idle during the ramp);
    later ones on DVE.  Block 0/1 chains are pre-issued before the pair
    stream with a short PE warm-up to reach full tensor-engine p-state.
"""

import sys

if "/opt/trn_rl_repo" not in sys.path:
    sys.path.insert(0, "/opt/trn_rl_repo")

import numpy as np

import concourse.bass as bass  # noqa: F401  (AP types used implicitly)
import concourse.tile as tile
from concourse import bacc, mybir
from concourse.bass import ds
from concourse.bass_utils import run_bass_kernel_spmd
from concourse.masks import make_identity

B = 8
T_FULL = 4096
C = 384
H = 64
P = 128
TQ = 512  # q-block width
SCALE = 1.0 / float(np.sqrt(C))
F32 = mybir.dt.float32

F16 = mybir.dt.float16
MM_DTYPE = F16  # matmul pipeline dtype (fp16: 1 cyc/row + fast weight load)


def build_nc(T=T_FULL, mm_dtype=MM_DTYPE):
    """Build the per-core Bass program (same program on all 8 cores)."""
    NT = T // P  # number of 128-row s-chunks
    NQ = T // TQ  # number of 512-row q-blocks
    CC = C // P  # 3 embed chunks
    SUB = TQ // P  # 4 sub-tiles per block

    MMD = mm_dtype  # tiles feeding matmuls are allocated in this dtype

    nc = bacc.Bacc(
        "TRN2",
        target_bir_lowering=False,
        debug=False,
        enable_asserts=True,
        num_devices=B,
    )
    x_ap = nc.dram_tensor("x", [T, C], F32, kind="ExternalInput").ap()
    wq_ap = nc.dram_tensor("Wq", [C, H], F32, kind="ExternalInput").ap()
    wk_ap = nc.dram_tensor("Wk", [C, H], F32, kind="ExternalInput").ap()
    wv_ap = nc.dram_tensor("Wv", [C, H], F32, kind="ExternalInput").ap()
    out_ap = nc.dram_tensor("out", [T, H], F32, kind="ExternalOutput").ap()

    x_re = x_ap.rearrange("(n p) c -> p n c", p=P)  # [128, NT, 384]
    out_re = out_ap.rearrange("(n p) h -> p n h", p=P)  # [128, NT, 64]

    with tile.TileContext(nc) as tc:
        with (
            tc.tile_pool(name="consts", bufs=1) as consts,
            tc.tile_pool(name="xnat", bufs=5) as xnat,
            tc.tile_pool(name="xtp", bufs=3) as xtp,
            tc.tile_pool(name="qkt", bufs=1) as qktp,
            tc.tile_pool(name="vextp", bufs=1) as vextp,
            tc.tile_pool(name="ptp", bufs=4) as ptp,
            tc.tile_pool(name="otp", bufs=2) as otp,
            tc.tile_pool(name="op", bufs=2) as op_,
            tc.tile_pool(name="rvp", bufs=2) as rvp,
            tc.tile_pool(name="psum", bufs=2, space="PSUM") as psum,
        ):
            ident_h = consts.tile([P, P], MMD)
            # packed QK weights: chunk c -> [Wq_c | Wk_c]  [128, 128]
            wqk_sb = consts.tile([P, CC, P], MMD)
            wv_sb = consts.tile([P, CC, H], MMD)
            qt2 = qktp.tile([H, T], MMD, tag="qt")
            kt2 = qktp.tile([H, T], MMD, tag="kt")
            vext = vextp.tile([P, NT, H + 1], MMD)
            ones_col = consts.tile([P, NT, 1], F32)
            # causal triangle mask: tri[s, q] = 1.0 where q >= s (128x128)
            tri = consts.tile([P, P], MMD)

            def emit_consts():
                make_identity(nc, ident_h)
                nc.gpsimd.dma_start(
                    out=wqk_sb[:, :, 0:H],
                    in_=wq_ap.rearrange("(c p) h -> p c h", p=P),
                )
                nc.gpsimd.dma_start(
                    out=wqk_sb[:, :, H:P],
                    in_=wk_ap.rearrange("(c p) h -> p c h", p=P),
                )
                nc.gpsimd.dma_start(
                    out=wv_sb, in_=wv_ap.rearrange("(c p) h -> p c h", p=P)
                )
                nc.vector.memset(ones_col, 1.0)
                nc.vector.tensor_copy(out=vext[:, :, H : H + 1], in_=ones_col)
                nc.vector.memset(tri, 1.0)
                # PE p-state warm-up: keep the tensor engine busy while the
                # first x tiles stream in, so real work starts at full clock
                warm = psum.tile([P, P], MMD, tag="small", name="warm")
                for _ in range(6):
                    nc.tensor.transpose(warm, ident_h, ident_h)
                nc.gpsimd.affine_select(
                    out=tri,
                    in_=tri,
                    compare_op=mybir.AluOpType.is_ge,
                    fill=0.0,
                    base=0,
                    pattern=[[1, P]],
                    channel_multiplier=-1,
                )

            def p1copy(j, out, in_, qk=False):
                # block 0's Q/K copies go on the still-idle ScalarE so DVE
                # can race ahead on the X^T copies (ramp latency).
                if qk and j <= 1:
                    nc.scalar.copy(out=out, in_=in_)
                else:
                    nc.vector.tensor_copy(out=out, in_=in_)

            def phase1_gen(j):
                """Load x rows [512j, 512j+512), produce X^T, Q^T, K^T, V.

                Yields between small PE chunks so the driver can spread
                this work into the gaps of the attention pair loop.
                """
                xn = xnat.tile([P, SUB, C], MMD, tag="xn", name=f"xn{j}")
                nc.gpsimd.dma_start(
                    out=xn[:, 0:2, :], in_=x_re[:, SUB * j : SUB * j + 2, :]
                )
                nc.gpsimd.dma_start(
                    out=xn[:, 2:4, :], in_=x_re[:, SUB * j + 2 : SUB * (j + 1), :]
                )
                xt = xtp.tile([P, CC, TQ], MMD, tag="xt", name=f"xt{j}")
                yield
                for st in range(SUB):
                    pst = psum.tile([P, CC, P], MMD, tag="small", name=f"pst{j}_{st}")
                    for c in range(CC):
                        nc.tensor.transpose(
                            pst[:, c, :], xn[:, st, c * P : (c + 1) * P], ident_h
                        )
                    p1copy(j, xt[:, :, st * P : (st + 1) * P], pst)
                    yield
                blk = ds(j * TQ, TQ)
                psqk = psum.tile([P, TQ], F32, tag="acc", name=f"psqk{j}")
                for c in range(CC):
                    nc.tensor.matmul(
                        psqk,
                        lhsT=wqk_sb[:, c, :],
                        rhs=xt[:, c, :],
                        start=(c == 0),
                        stop=(c == CC - 1),
                    )
                p1copy(j, qt2[:, blk], psqk[0:H, :], qk=True)
                yield
                p1copy(j, kt2[:, blk], psqk[H:P, :], qk=True)
                yield
                for st in range(SUB):
                    psvn = psum.tile([P, H], F32, tag="acc", name=f"psvn{j}_{st}")
                    for c in range(CC):
                        nc.tensor.matmul(
                            psvn,
                            lhsT=xt[:, c, st * P : (st + 1) * P],
                            rhs=wv_sb[:, c, :],
                            start=(c == 0),
                            stop=(c == CC - 1),
                        )
                    nc.vector.tensor_copy(
                        out=vext[:, SUB * j + st, 0:H], in_=psvn
                    )
                    yield

            N1_CHUNKS = 11  # number of yields in phase1_gen

            def make_finish(j, psoj):
                def finish(last=(j == NQ - 1)):
                    ot = otp.tile([H + 1, TQ], MMD, tag="ot", name=f"ot{j}")
                    if last:
                        # ScalarE is idle once the final exp retires
                        nc.scalar.copy(out=ot, in_=psoj)
                    else:
                        nc.vector.tensor_copy(out=ot, in_=psoj)
                    pstr = psum.tile(
                        [P, SUB, H + 2], MMD, tag="small", name=f"pstr{j}"
                    )
                    for i in range(SUB):
                        nc.tensor.transpose(
                            pstr[:, i, 0 : H + 1],
                            ot[:, i * P : (i + 1) * P],
                            ident_h[0 : H + 1, 0 : H + 1],
                        )
                    o = op_.tile([P, SUB, H + 1], F32, tag="o", name=f"o{j}")
                    nc.vector.tensor_copy(out=o, in_=pstr[:, :, 0 : H + 1])
                    rv = rvp.tile([P, SUB], F32, tag="rv", name=f"rv{j}")
                    nc.vector.reciprocal(out=rv, in_=o[:, :, H : H + 1])
                    for i in range(SUB):
                        eng = nc.gpsimd if (last and i >= 2) else nc.vector
                        eng.tensor_scalar_mul(
                            out=o[:, i, 0:H],
                            in0=o[:, i, 0:H],
                            scalar1=rv[:, i : i + 1],
                        )
                    if last:
                        # final block: split the store across two queues so
                        # the tail isn't serialized behind one DMA
                        nc.sync.dma_start(
                            out=out_re[:, SUB * j : SUB * j + 2, :],
                            in_=o[:, 0:2, 0:H],
                        )
                        nc.scalar.dma_start(
                            out=out_re[:, SUB * j + 2 : SUB * (j + 1), :],
                            in_=o[:, 2:4, 0:H],
                        )
                    else:
                        nc.sync.dma_start(
                            out=out_re[:, SUB * j : SUB * (j + 1), :],
                            in_=o[:, :, 0:H],
                        )

                return finish

            # ---- Global chunk-stream attention ----
            # Every (block, s-chunk) score/exp/PV unit joins one continuous
            # stream of chunk PAIRS; a pair may span a block boundary, so
            # ScalarE sees a gapless exp stream with no per-block drain.
            # pso accumulators of adjacent blocks overlap (acc tag, 2 bufs)
            # and each block's output path is deferred by one pair so it
            # never stalls the following block's score matmuls.
            chunks = [(j, c) for j in range(NQ) for c in range(SUB * (j + 1))]
            pairs = [tuple(chunks[i : i + 2]) for i in range(0, len(chunks), 2)]
            pair_block = [pr[0][0] for pr in pairs]
            group_start = {}
            group_len = {}
            for pi, b in enumerate(pair_block):
                group_start.setdefault(b, pi)
                group_len[b] = group_len.get(b, 0) + 1

            gens = [phase1_gen(j) for j in range(NQ)]
            left = [N1_CHUNKS] * NQ  # chunks remaining per generator

            def advance(g):
                if g >= NQ or left[g] <= 0:
                    return
                try:
                    next(gens[g])
                except StopIteration:
                    left[g] = 0
                    return
                left[g] -= 1

            def drain(g):
                while g < NQ and left[g] > 0:
                    advance(g)

            advance(0)  # issue block 0's x DMA before const init
            emit_consts()
            advance(1)  # pre-issue the next three blocks' x DMAs
            advance(2)
            advance(3)
            drain(0)
            # pre-advance the next blocks' transpose chains: their x data
            # arrives during block 0's first pairs, and PE has slack there
            while left[1] > N1_CHUNKS - 8:
                advance(1)
            while left[2] > N1_CHUNKS - 5:
                advance(2)

            pso = {}
            pending = [None]
            req_quota = {}  # req gen chunk count at its pacing-group start
            pstate = {"pi": 0}

            def _pace(budget):
                # advance pending phase-1 toward its deadline, at most
                # `budget` chunks (keeps PE bursts small so score matmuls
                # are never delayed long, which would starve ScalarE)
                pi = pstate["pi"]
                cur = pair_block[pi]
                req, opp = cur + 1, cur + 2
                for g in (req, opp):
                    if g < NQ and left[g] == N1_CHUNKS:
                        advance(g)
                done = pi - group_start[cur] + 1
                total = group_len[cur]
                if req < NQ and left[req] > 0:
                    if req not in req_quota:
                        req_quota[req] = left[req]
                    want = -(-req_quota[req] * done // total)  # ceil
                    while (
                        left[req] > req_quota[req] - want
                        and left[req] > 0
                        and budget > 0
                    ):
                        advance(req)
                        budget -= 1
                if opp < NQ and left[opp] > 0 and budget > 0:
                    if opp not in req_quota:
                        req_quota[opp] = left[opp]
                    want = req_quota[opp] * done // (2 * total)
                    while (
                        left[opp] > req_quota[opp] - want
                        and left[opp] > 0
                        and budget > 0
                    ):
                        advance(opp)
                        budget -= 1
                for lvl, g in enumerate(range(opp + 1, min(opp + 3, NQ))):
                    if left[g] > 0 and budget > 0:
                        if g not in req_quota:
                            req_quota[g] = left[g]
                        want = req_quota[g] * done // ((4 << lvl) * total)
                        while (
                            left[g] > req_quota[g] - want
                            and left[g] > 0
                            and budget > 0
                        ):
                            advance(g)
                            budget -= 1

            def pump1():
                _pace(4)

            def pump2():
                _pace(6)

            for pi, pair in enumerate(pairs):
                pstate["pi"] = pi
                cur = pair_block[pi]
                for j, c in pair:
                    if c == 0:
                        drain(j)  # phase-1 of block j must be complete
                        pso[j] = psum.tile(
                            [H + 1, TQ], F32, tag="acc", name=f"pso{j}"
                        )
                pss = psum.tile([P, 2 * TQ], F32, tag="wide", name=f"pss{pi}")
                geoms = []
                base = 0
                for j, c in pair:
                    d = c - SUB * j
                    off = P * d if d >= 0 else 0
                    w = TQ - off
                    nc.tensor.matmul(
                        pss[:, base : base + w],
                        lhsT=kt2[:, c * P : (c + 1) * P],
                        rhs=qt2[:, j * TQ + off : (j + 1) * TQ],
                        start=True,
                        stop=True,
                    )
                    geoms.append((j, c, off, w, base, d >= 0))
                    base += w
                pt = ptp.tile([P, 2 * TQ], MMD, tag="pt", name=f"pt{pi}")
                nc.scalar.activation(
                    out=pt[:, 0:base],
                    in_=pss[:, 0:base],
                    func=mybir.ActivationFunctionType.Exp,
                    scale=SCALE,
                )
                # causal triangle mask on the leading 128 cols of each
                # diagonal chunk's region (keep where q_local >= s_local)
                for j, c, off, w, b, diag in geoms:
                    if diag:
                        nc.vector.tensor_mul(
                            out=pt[:, b : b + P],
                            in0=pt[:, b : b + P],
                            in1=tri,
                        )
                pump1()
                newly_done = None
                for j, c, off, w, b, diag in geoms:
                    nch = SUB * (j + 1)
                    nc.tensor.matmul(
                        pso[j][:, off:TQ],
                        lhsT=vext[:, c, :],
                        rhs=pt[:, b : b + w],
                        start=(c == 0),
                        stop=(c == nch - 1),
                    )
                    if c == nch - 1:
                        newly_done = j
                # emit the one-pair-deferred output path BEFORE pumping, so
                # its pso buffer frees before pumped phase-1 wants an acc slot
                if pending[0] is not None:
                    pending[0]()
                    pending[0] = None
                if newly_done is not None:
                    pending[0] = make_finish(newly_done, pso.pop(newly_done))
                # pace phase-1: gen cur+1 must finish by its group start,
                # gen cur+2 advances at half rate (deferred to pump2 below)
                pump2()
            if pending[0] is not None:
                pending[0]()

    nc.compile()
    return nc


_NC_CACHE = {}


def _get_nc():
    if "nc" not in _NC_CACHE:
        _NC_CACHE["nc"] = build_nc()
    return _NC_CACHE["nc"]


def kernel(x, Wk, Wq, Wv, _trace=False, _trace_kwargs=None):
    x = np.ascontiguousarray(x, dtype=np.float32)
    Wk = np.ascontiguousarray(Wk, dtype=np.float32)
    Wq = np.ascontiguousarray(Wq, dtype=np.float32)
    Wv = np.ascontiguousarray(Wv, dtype=np.float32)
    nc = _get_nc()
    in_maps = [
        {"x": x[b], "Wq": Wq, "Wk": Wk, "Wv": Wv} for b in range(B)
    ]
    res = run_bass_kernel_spmd(
        nc, in_maps, list(range(B)), trace=_trace, **(_trace_kwargs or {})
    )
    out = np.stack([res.results[b]["out"] for b in range(B)], axis=0)
    if _trace:
        return out, res
    return out
